# revision 23
# baseline (speedup 1.0000x reference)
"""CQAttention (QANet context-query attention) Trainium2 kernel.

Problem: B=64, H=256, Lc=2048, Lq=256.
  S[b,i,j] = (Ct@w1)[i] + (Qt@w2)[j] + sum_h Ct[i,h]*w3[h]*Qt[j,h]
  S_row = softmax_j(masked), S_col = softmax_i(masked)
  A = S_row @ Qt ; Bt = S_row @ (S_col^T @ Ct)
  out[b] = [Ct; A; Ct*A; Ct*Bt]^T  -> [B, 4H, Lc]

Strategy: data-parallel over batch (8 per core x 8 cores).
  - section 0 of the output is exactly the input C -> host-assembled.
  - sections 2,3 are elementwise C*A / C*Bt -> computed on host from the
    device A/Bt. Device writes only A^T and Bt^T as fp16 (16MB/core).
  - row path (feeds A directly) stays fp16. The col path S3 matmul, the
    exp'd col weights Pc, Ct and the X=Pc^T@[Ct|1] matmul are all fp8
    e4m3 in DoubleRow mode (K=256/PE pass): col-softmax output is doubly
    averaged before reaching the output so fp8 noise washes out there.
    sqrt(|w3|) folded into both S3 operands balances fp8 range; the 4x4
    gain is undone by the ACT exp scale (1/16). Pc carries a -ln(64)
    bias shift so exp fits e4m3's 240 max (cancels in col-normalize).
  - rowsums via ones-matmul replicated across partitions, software-
    pipelined one tile behind S^T so the PE never waits on ACT.
  - per-batch order col -> X -> row -> M2 -> M4: the fp8 col operands are
    4x smaller, so batch 0's PE work starts while the fp16 C streams in.
  - softmax normalization fused into the PSUM evictions (DVE); one merged
    output DMA per 128-row section.
"""

import numpy as np

B, H, LC, LQ = 64, 256, 2048, 256
NCORES = 8
NB = B // NCORES  # batches per core
NEG = 1.0e30

HC = H // 128   # 2 h-chunks
JC = LQ // 128  # 2 j-chunks
IC = LC // 128  # 16 i-chunks
IT = LC // 512  # 4 i-tiles
HA = H + 1      # augmented (ones column) width

_CACHE = {}


def _build():
    import concourse.bacc as bacc
    import concourse.mybir as mybir
    import concourse.tile as tile
    from contextlib import ExitStack

    F32 = mybir.dt.float32
    F16 = mybir.dt.float16
    F8 = mybir.dt.float8e4
    AF = mybir.ActivationFunctionType
    MUL = mybir.AluOpType.mult
    DR = mybir.MatmulPerfMode.DoubleRow

    nc = bacc.Bacc("TRN2", target_bir_lowering=False, debug=False,
                   enable_asserts=False)

    c16 = nc.dram_tensor("c16", [NB, 128, HC * LC], F16, kind="ExternalInput").ap()
    q3 = nc.dram_tensor("q3", [NB, 128, HC * LQ], F16, kind="ExternalInput").ap()
    c8 = nc.dram_tensor("c8", [NB, 128, HC * LC], F8, kind="ExternalInput").ap()
    cta = nc.dram_tensor("cta", [NB, 128, IC * HA], F8, kind="ExternalInput").ap()
    q38 = nc.dram_tensor("q38", [NB, 128, HC * LQ], F8, kind="ExternalInput").ap()
    qt = nc.dram_tensor("qt", [NB, 128, JC * H], F16, kind="ExternalInput").ap()
    rcb = nc.dram_tensor("rcb", [NB, 128, IC + JC], F32, kind="ExternalInput").ap()
    out = nc.dram_tensor("out", [NB, 2 * H, LC], F16, kind="ExternalOutput").ap()

    with tile.TileContext(nc) as tc:
        with ExitStack() as ctx:
            konst = ctx.enter_context(tc.tile_pool(name="konst", bufs=1))
            crpool = ctx.enter_context(tc.tile_pool(name="crpool", bufs=2))
            ctpool = ctx.enter_context(tc.tile_pool(name="ctpool", bufs=2))
            qpool = ctx.enter_context(tc.tile_pool(name="qpool", bufs=3))
            prpool = ctx.enter_context(tc.tile_pool(name="prpool", bufs=2))
            pcpool = ctx.enter_context(tc.tile_pool(name="pcpool", bufs=2))
            rrpool = ctx.enter_context(tc.tile_pool(name="rrpool", bufs=2))
            xpool = ctx.enter_context(tc.tile_pool(name="xpool", bufs=2))
            opool = ctx.enter_context(tc.tile_pool(name="opool", bufs=3))
            small = ctx.enter_context(tc.tile_pool(name="small", bufs=6))
            mm_ps = ctx.enter_context(tc.tile_pool(name="mm_ps", bufs=5, space="PSUM"))
            s3_ps = ctx.enter_context(tc.tile_pool(name="s3_ps", bufs=2, space="PSUM"))
            x_ps = ctx.enter_context(tc.tile_pool(name="x_ps", bufs=1, space="PSUM"))

            ones32 = konst.tile([128, 128], F32)
            nc.vector.memset(ones32[:], 1.0)
            ones16 = konst.tile([128, 128], F16)
            nc.vector.tensor_copy(ones16[:], ones32[:])

            def load_batch(b):
                # fp8 col-path operands first (small): batch 0's PE work can
                # start on the col path while the big fp16 C streams in.
                # Batch 0 splits the big loads so the first tiles land sooner;
                # steady-state batches use single DMAs (fewer Sync issues).
                split = b == 0
                q38sb = qpool.tile([128, HC * LQ], F8, tag="q38sb")
                nc.sync.dma_start(q38sb[:], q38[b])
                rcbsb = small.tile([128, IC + JC], F32, tag="rcbsb")
                nc.sync.dma_start(rcbsb[:], rcb[b])
                c8sb = crpool.tile([128, HC * LC], F8, tag="c8sb")
                if split:
                    c83 = c8sb[:].rearrange("p (c i) -> p c i", c=HC)
                    c8d = c8[b].rearrange("p (c i) -> p c i", c=HC)
                    nc.sync.dma_start(c83[:, :, 0:1024], c8d[:, :, 0:1024])
                    nc.sync.dma_start(c83[:, :, 1024:2048], c8d[:, :, 1024:2048])
                else:
                    nc.sync.dma_start(c8sb[:], c8[b])
                ctsb = ctpool.tile([128, IC * HA], F8, tag="ctsb")
                nc.sync.dma_start(ctsb[:], cta[b])
                # batch 0: issue the fp16 row-path loads from the scalar HWDGE
                # queue in parallel with the sync queue (idle at the head)
                eng = nc.scalar if split else nc.sync
                q3sb = qpool.tile([128, HC * LQ], F16, tag="q3sb")
                eng.dma_start(q3sb[:], q3[b])
                crsb = crpool.tile([128, HC * LC], F16, tag="crsb")
                if split:
                    cr3 = crsb[:].rearrange("p (c i) -> p c i", c=HC)
                    cd3 = c16[b].rearrange("p (c i) -> p c i", c=HC)
                    eng.dma_start(cr3[:, :, 0:1024], cd3[:, :, 0:1024])
                    eng.dma_start(cr3[:, :, 1024:2048], cd3[:, :, 1024:2048])
                else:
                    eng.dma_start(crsb[:], c16[b])
                qtsb = qpool.tile([128, JC * H], F16, tag="qtsb")
                eng.dma_start(qtsb[:], qt[b])
                return crsb, q3sb, c8sb, ctsb, q38sb, qtsb, rcbsb

            tiles = load_batch(0)
            for b in range(NB):
                crsb, q3sb, c8sb, ctsb, q38sb, qtsb, rcbsb = tiles
                rmsb = rcbsb[:, 0:IC]
                cbsb = rcbsb[:, IC:IC + JC]
                if b + 1 < NB:
                    tiles = load_batch(b + 1)

                c83 = c8sb[:].rearrange("p (c i) -> p c i", c=HC)
                q383 = q38sb[:].rearrange("p (c j) -> p c j", c=HC)

                # ---- col path first: S (fp8 DR) -> exp -> Pc (fp8).
                # Its operands are 4x smaller than the fp16 row-path ones, so
                # batch 0's PE work starts while the big fp16 C streams in.
                pc = pcpool.tile([128, IC * LQ], F8, tag="pc")
                for ic in range(IC):
                    ps3 = s3_ps.tile([128, LQ], F32, tag="s3")
                    nc.tensor.matmul(
                        ps3[:],
                        c83[:, :, ic * 128:(ic + 1) * 128],
                        q383[:, :, :],
                        start=True, stop=True, perf_mode=DR)
                    nc.scalar.activation(
                        pc[:, ic * LQ:(ic + 1) * LQ],
                        ps3[:], AF.Exp, bias=rmsb[:, ic:ic + 1], scale=1.0 / 16.0)
                pc3 = pc[:].rearrange("p (n j) -> p n j", n=IC)
                ct3 = ctsb[:].rearrange("p (n h) -> p n h", n=IC)

                # ---- M3: X_aug = Pc^T @ [Ct|1] (fp8 DR over ic pairs) ----
                def x_block():
                    xsb = xpool.tile([128, JC * H], F16, tag="xsb")
                    for jc in range(JC):
                        xps = x_ps.tile([128, HA], F32, tag="x")
                        for g in range(IC // 2):
                            nc.tensor.matmul(
                                xps[:],
                                pc3[:, 2 * g:2 * g + 2, jc * 128:(jc + 1) * 128],
                                ct3[:, 2 * g:2 * g + 2, :],
                                start=(g == 0), stop=(g == IC // 2 - 1),
                                perf_mode=DR)
                        colr = small.tile([128, 1], F32, tag="colr")
                        nc.vector.reciprocal_approx_fast(colr[:], xps[:, H:H + 1])
                        nc.vector.tensor_scalar_mul(
                            xsb[:, jc * H:(jc + 1) * H], xps[:, 0:H], colr[:])
                    return xsb

                # steady state: X right after col (its exps ran during the
                # previous batch's M2/M4). Batch 0 has no such overlap, so X
                # would stall on the col exps there -- run it after the row
                # path instead.
                xsb = x_block() if b > 0 else None

                # ---- row path: S^T (fp16) -> exp -> Pr^T; pipelined rowsums ----
                prt = prpool.tile([128, JC * LC], F16, tag="prt")
                rrep = rrpool.tile([128, LC], F32, tag="rrep")

                def rowsum(it):
                    rs = mm_ps.tile([128, 512], F32, tag="mm")
                    for jc in range(JC):
                        nc.tensor.matmul(
                            rs[:], ones16[:],
                            prt[:, jc * LC + it * 512:jc * LC + (it + 1) * 512],
                            start=(jc == 0), stop=(jc == JC - 1))
                    nc.vector.reciprocal_approx_fast(
                        rrep[:, it * 512:(it + 1) * 512], rs[:])

                for it in range(IT):
                    for jc in range(JC):
                        ps = mm_ps.tile([128, 512], F32, tag="mm")
                        for kc in range(HC):
                            nc.tensor.matmul(
                                ps[:],
                                q3sb[:, kc * LQ + jc * 128:kc * LQ + (jc + 1) * 128],
                                crsb[:, kc * LC + it * 512:kc * LC + (it + 1) * 512],
                                start=(kc == 0), stop=(kc == HC - 1))
                        nc.scalar.activation(
                            prt[:, jc * LC + it * 512:jc * LC + (it + 1) * 512],
                            ps[:], AF.Exp, bias=cbsb[:, jc:jc + 1])
                    if it > 0:
                        rowsum(it - 1)
                rowsum(IT - 1)

                if xsb is None:
                    xsb = x_block()

                # ---- M2/M4: A^T, Bt^T; evict-normalize on DVE. The last
                # batch splits its DMAs so the final transfer starts sooner.
                def mout(row0, lhs, sec):
                    for hc in range(HC):
                        o = opool.tile([128, LC], F16, tag=f"o{sec}")
                        for it in range(IT):
                            i0, i1 = it * 512, (it + 1) * 512
                            ps_o = mm_ps.tile([128, 512], F32, tag="mm")
                            for jc in range(JC):
                                nc.tensor.matmul(
                                    ps_o[:],
                                    lhs[:, jc * H + hc * 128:jc * H + (hc + 1) * 128],
                                    prt[:, jc * LC + i0:jc * LC + i1],
                                    start=(jc == 0), stop=(jc == JC - 1))
                            nc.vector.tensor_tensor(
                                o[:, i0:i1], ps_o[:], rrep[:, i0:i1], MUL)
                            if b == NB - 1 and it % 2 == 1:
                                nc.sync.dma_start(
                                    out[b, row0 + hc * 128:row0 + (hc + 1) * 128,
                                        i0 - 512:i1], o[:, i0 - 512:i1])
                        if b < NB - 1:
                            nc.sync.dma_start(
                                out[b, row0 + hc * 128:row0 + (hc + 1) * 128, :],
                                o[:])

                mout(0, qtsb, "a")
                mout(H, xsb, "b")

    nc.compile()
    return nc


def _prep(C, Q, cmask, qmask, line_project):
    import ml_dtypes
    w1, w2, w3 = np.split(line_project.astype(np.float64), 3)
    r = np.einsum('bhi,h->bi', C.astype(np.float64), w1).astype(np.float32)
    c_ = np.einsum('bhj,h->bj', Q.astype(np.float64), w2).astype(np.float32)
    # -ln(64) shift keeps exp within fp8 e4m3 range; cancels in col-normalize
    rm = (r - NEG * cmask - np.float32(np.log(64.0))).reshape(
        B, IC, 128).transpose(0, 2, 1)
    cb = (c_ - NEG * qmask).reshape(B, JC, 128).transpose(0, 2, 1)
    rcb = np.concatenate([rm, cb], axis=2).astype(np.float32)

    # fp16 row-path operands
    c16 = np.ascontiguousarray(
        C.reshape(B, HC, 128, LC).transpose(0, 2, 1, 3)).astype(np.float16)
    w3f = w3.astype(np.float32)
    q3v = Q * w3f[None, :, None]
    q3 = np.ascontiguousarray(
        q3v.reshape(B, HC, 128, LQ).transpose(0, 2, 1, 3)).astype(np.float16)

    # fp8 col-path operands: fold 4*sqrt(|w3|) into both sides;
    # S3_dev = 16*S3, undone by the ACT exp scale (1/16).
    sq = 4.0 * np.sqrt(np.abs(w3f))
    c8v = C * sq[None, :, None]
    c8 = np.ascontiguousarray(
        c8v.reshape(B, HC, 128, LC).transpose(0, 2, 1, 3)
    ).astype(ml_dtypes.float8_e4m3)
    q38v = Q * (np.sign(w3f) * sq)[None, :, None]
    q38 = np.ascontiguousarray(
        q38v.reshape(B, HC, 128, LQ).transpose(0, 2, 1, 3)
    ).astype(ml_dtypes.float8_e4m3)

    Ct = C.transpose(0, 2, 1)  # [B, LC, H]
    cta = np.ones((B, 128, IC, HA), dtype=ml_dtypes.float8_e4m3)
    cta[..., :H] = Ct.reshape(B, IC, 128, H).transpose(0, 2, 1, 3).astype(
        ml_dtypes.float8_e4m3)
    qt = np.ascontiguousarray(
        Q.transpose(0, 2, 1).reshape(B, JC, 128, H).transpose(0, 2, 1, 3)
    ).astype(np.float16)
    return rcb, c16, q3, c8, cta, q38, qt


def make_in_maps(C, Q, cmask, qmask, line_project):
    C = np.asarray(C, dtype=np.float32)
    Q = np.asarray(Q, dtype=np.float32)
    cmask = np.asarray(cmask, dtype=np.float32)
    qmask = np.asarray(qmask, dtype=np.float32)
    line_project = np.asarray(line_project, dtype=np.float32)
    rcb, c16, q3, c8, cta, q38, qt = _prep(C, Q, cmask, qmask, line_project)
    in_maps = []
    for core in range(NCORES):
        s = slice(core * NB, (core + 1) * NB)
        in_maps.append({
            "c16": np.ascontiguousarray(c16[s]).reshape(NB, 128, HC * LC),
            "q3": np.ascontiguousarray(q3[s]).reshape(NB, 128, HC * LQ),
            "c8": np.ascontiguousarray(c8[s]).reshape(NB, 128, HC * LC),
            "cta": np.ascontiguousarray(cta[s]).reshape(NB, 128, IC * HA),
            "q38": np.ascontiguousarray(q38[s]).reshape(NB, 128, HC * LQ),
            "qt": np.ascontiguousarray(qt[s]).reshape(NB, 128, JC * H),
            "rcb": np.ascontiguousarray(rcb[s]),
        })
    return in_maps


def kernel(C, Q, cmask, qmask, line_project):
    from concourse.bass_utils import run_bass_kernel_spmd

    C = np.asarray(C, dtype=np.float32)
    in_maps = make_in_maps(C, Q, cmask, qmask, line_project)
    if "nc" not in _CACHE:
        _CACHE["nc"] = _build()
    nc = _CACHE["nc"]
    res = run_bass_kernel_spmd(nc, in_maps, core_ids=list(range(NCORES)))
    _CACHE["last_results"] = res
    dev = np.concatenate([res.results[c]["out"] for c in range(NCORES)], axis=0)
    A = dev[:, :H].astype(np.float32)
    Bt = dev[:, H:].astype(np.float32)
    full = np.empty((B, 4 * H, LC), dtype=np.float32)
    full[:, :H] = C
    full[:, H:2 * H] = A
    full[:, 2 * H:3 * H] = C * A
    full[:, 3 * H:] = C * Bt
    return full


# revision 24
# speedup vs baseline: 1.0078x; 1.0078x over previous
"""CQAttention (QANet context-query attention) Trainium2 kernel.

Problem: B=64, H=256, Lc=2048, Lq=256.
  S[b,i,j] = (Ct@w1)[i] + (Qt@w2)[j] + sum_h Ct[i,h]*w3[h]*Qt[j,h]
  S_row = softmax_j(masked), S_col = softmax_i(masked)
  A = S_row @ Qt ; Bt = S_row @ (S_col^T @ Ct)
  out[b] = [Ct; A; Ct*A; Ct*Bt]^T  -> [B, 4H, Lc]

Strategy: data-parallel over batch (8 per core x 8 cores).
  - section 0 of the output is exactly the input C -> host-assembled.
  - sections 2,3 are elementwise C*A / C*Bt -> computed on host from the
    device A/Bt. Device writes only A^T and Bt^T as fp16 (16MB/core).
  - row path (feeds A directly) stays fp16. The col path S3 matmul, the
    exp'd col weights Pc, Ct and the X=Pc^T@[Ct|1] matmul are all fp8
    e4m3 in DoubleRow mode (K=256/PE pass): col-softmax output is doubly
    averaged before reaching the output so fp8 noise washes out there.
    sqrt(|w3|) folded into both S3 operands balances fp8 range; the 4x4
    gain is undone by the ACT exp scale (1/16). Pc carries a -ln(64)
    bias shift so exp fits e4m3's 240 max (cancels in col-normalize).
  - rowsums via ones-matmul replicated across partitions, software-
    pipelined one tile behind S^T so the PE never waits on ACT.
  - per-batch order col -> X -> row -> M2 -> M4: the fp8 col operands are
    4x smaller, so batch 0's PE work starts while the fp16 C streams in.
  - softmax normalization fused into the PSUM evictions (DVE); one merged
    output DMA per 128-row section.
"""

import numpy as np

B, H, LC, LQ = 64, 256, 2048, 256
NCORES = 8
NB = B // NCORES  # batches per core
NEG = 1.0e30

HC = H // 128   # 2 h-chunks
JC = LQ // 128  # 2 j-chunks
IC = LC // 128  # 16 i-chunks
IT = LC // 512  # 4 i-tiles
HA = H + 1      # augmented (ones column) width

_CACHE = {}


def _build():
    import concourse.bacc as bacc
    import concourse.mybir as mybir
    import concourse.tile as tile
    from contextlib import ExitStack

    F32 = mybir.dt.float32
    F16 = mybir.dt.float16
    F8 = mybir.dt.float8e4
    AF = mybir.ActivationFunctionType
    MUL = mybir.AluOpType.mult
    DR = mybir.MatmulPerfMode.DoubleRow

    nc = bacc.Bacc("TRN2", target_bir_lowering=False, debug=False,
                   enable_asserts=False)

    c16 = nc.dram_tensor("c16", [NB, 128, HC * LC], F16, kind="ExternalInput").ap()
    q3 = nc.dram_tensor("q3", [NB, 128, HC * LQ], F16, kind="ExternalInput").ap()
    c8 = nc.dram_tensor("c8", [NB, 128, HC * LC], F8, kind="ExternalInput").ap()
    cta = nc.dram_tensor("cta", [NB, 128, IC * HA], F8, kind="ExternalInput").ap()
    q38 = nc.dram_tensor("q38", [NB, 128, HC * LQ], F8, kind="ExternalInput").ap()
    qt = nc.dram_tensor("qt", [NB, 128, JC * H], F16, kind="ExternalInput").ap()
    rcb = nc.dram_tensor("rcb", [NB, 128, IC + JC], F32, kind="ExternalInput").ap()
    out = nc.dram_tensor("out", [NB, 2 * H, LC], F16, kind="ExternalOutput").ap()

    with tile.TileContext(nc) as tc:
        with ExitStack() as ctx:
            konst = ctx.enter_context(tc.tile_pool(name="konst", bufs=1))
            crpool = ctx.enter_context(tc.tile_pool(name="crpool", bufs=2))
            ctpool = ctx.enter_context(tc.tile_pool(name="ctpool", bufs=2))
            qpool = ctx.enter_context(tc.tile_pool(name="qpool", bufs=3))
            prpool = ctx.enter_context(tc.tile_pool(name="prpool", bufs=2))
            pcpool = ctx.enter_context(tc.tile_pool(name="pcpool", bufs=2))
            rrpool = ctx.enter_context(tc.tile_pool(name="rrpool", bufs=2))
            xpool = ctx.enter_context(tc.tile_pool(name="xpool", bufs=2))
            opool = ctx.enter_context(tc.tile_pool(name="opool", bufs=3))
            small = ctx.enter_context(tc.tile_pool(name="small", bufs=6))
            mm_ps = ctx.enter_context(tc.tile_pool(name="mm_ps", bufs=5, space="PSUM"))
            s3_ps = ctx.enter_context(tc.tile_pool(name="s3_ps", bufs=2, space="PSUM"))
            x_ps = ctx.enter_context(tc.tile_pool(name="x_ps", bufs=1, space="PSUM"))

            ones32 = konst.tile([128, 128], F32)
            nc.vector.memset(ones32[:], 1.0)
            ones16 = konst.tile([128, 128], F16)
            nc.vector.tensor_copy(ones16[:], ones32[:])

            def load_batch(b):
                # fp8 col-path operands first (small): batch 0's PE work can
                # start on the col path while the big fp16 C streams in.
                # Batch 0 splits the big loads so the first tiles land sooner;
                # steady-state batches use single DMAs (fewer Sync issues).
                split = b == 0
                q38sb = qpool.tile([128, HC * LQ], F8, tag="q38sb")
                nc.sync.dma_start(q38sb[:], q38[b])
                rcbsb = small.tile([128, IC + JC], F32, tag="rcbsb")
                nc.sync.dma_start(rcbsb[:], rcb[b])
                c8sb = crpool.tile([128, HC * LC], F8, tag="c8sb")
                if split:
                    c83 = c8sb[:].rearrange("p (c i) -> p c i", c=HC)
                    c8d = c8[b].rearrange("p (c i) -> p c i", c=HC)
                    nc.sync.dma_start(c83[:, :, 0:1024], c8d[:, :, 0:1024])
                    nc.sync.dma_start(c83[:, :, 1024:2048], c8d[:, :, 1024:2048])
                else:
                    nc.sync.dma_start(c8sb[:], c8[b])
                ctsb = ctpool.tile([128, IC * HA], F8, tag="ctsb")
                nc.sync.dma_start(ctsb[:], cta[b])
                q3sb = qpool.tile([128, HC * LQ], F16, tag="q3sb")
                nc.sync.dma_start(q3sb[:], q3[b])
                crsb = crpool.tile([128, HC * LC], F16, tag="crsb")
                if split:
                    cr3 = crsb[:].rearrange("p (c i) -> p c i", c=HC)
                    cd3 = c16[b].rearrange("p (c i) -> p c i", c=HC)
                    nc.sync.dma_start(cr3[:, :, 0:1024], cd3[:, :, 0:1024])
                    nc.sync.dma_start(cr3[:, :, 1024:2048], cd3[:, :, 1024:2048])
                else:
                    nc.sync.dma_start(crsb[:], c16[b])
                qtsb = qpool.tile([128, JC * H], F16, tag="qtsb")
                nc.sync.dma_start(qtsb[:], qt[b])
                return crsb, q3sb, c8sb, ctsb, q38sb, qtsb, rcbsb

            tiles = load_batch(0)
            for b in range(NB):
                crsb, q3sb, c8sb, ctsb, q38sb, qtsb, rcbsb = tiles
                rmsb = rcbsb[:, 0:IC]
                cbsb = rcbsb[:, IC:IC + JC]
                if b + 1 < NB:
                    tiles = load_batch(b + 1)

                c83 = c8sb[:].rearrange("p (c i) -> p c i", c=HC)
                q383 = q38sb[:].rearrange("p (c j) -> p c j", c=HC)

                # ---- col path first: S (fp8 DR) -> exp -> Pc (fp8).
                # Its operands are 4x smaller than the fp16 row-path ones, so
                # batch 0's PE work starts while the big fp16 C streams in.
                pc = pcpool.tile([128, IC * LQ], F8, tag="pc")
                for ic in range(IC):
                    ps3 = s3_ps.tile([128, LQ], F32, tag="s3")
                    nc.tensor.matmul(
                        ps3[:],
                        c83[:, :, ic * 128:(ic + 1) * 128],
                        q383[:, :, :],
                        start=True, stop=True, perf_mode=DR)
                    nc.scalar.activation(
                        pc[:, ic * LQ:(ic + 1) * LQ],
                        ps3[:], AF.Exp, bias=rmsb[:, ic:ic + 1], scale=1.0 / 16.0)
                pc3 = pc[:].rearrange("p (n j) -> p n j", n=IC)
                ct3 = ctsb[:].rearrange("p (n h) -> p n h", n=IC)

                # ---- M3: X_aug = Pc^T @ [Ct|1] (fp8 DR over ic pairs) ----
                def x_block():
                    xsb = xpool.tile([128, JC * H], F16, tag="xsb")
                    for jc in range(JC):
                        xps = x_ps.tile([128, HA], F32, tag="x")
                        for g in range(IC // 2):
                            nc.tensor.matmul(
                                xps[:],
                                pc3[:, 2 * g:2 * g + 2, jc * 128:(jc + 1) * 128],
                                ct3[:, 2 * g:2 * g + 2, :],
                                start=(g == 0), stop=(g == IC // 2 - 1),
                                perf_mode=DR)
                        colr = small.tile([128, 1], F32, tag="colr")
                        nc.vector.reciprocal_approx_fast(colr[:], xps[:, H:H + 1])
                        nc.vector.tensor_scalar_mul(
                            xsb[:, jc * H:(jc + 1) * H], xps[:, 0:H], colr[:])
                    return xsb

                # steady state: X right after col (its exps ran during the
                # previous batch's M2/M4). Batch 0 has no such overlap, so X
                # would stall on the col exps there -- run it after the row
                # path instead.
                xsb = x_block() if b > 0 else None

                # ---- row path: S^T (fp16) -> exp -> Pr^T; pipelined rowsums ----
                prt = prpool.tile([128, JC * LC], F16, tag="prt")
                rrep = rrpool.tile([128, LC], F32, tag="rrep")

                def rowsum(it):
                    rs = mm_ps.tile([128, 512], F32, tag="mm")
                    for jc in range(JC):
                        nc.tensor.matmul(
                            rs[:], ones16[:],
                            prt[:, jc * LC + it * 512:jc * LC + (it + 1) * 512],
                            start=(jc == 0), stop=(jc == JC - 1))
                    nc.vector.reciprocal_approx_fast(
                        rrep[:, it * 512:(it + 1) * 512], rs[:])

                for it in range(IT):
                    for jc in range(JC):
                        ps = mm_ps.tile([128, 512], F32, tag="mm")
                        for kc in range(HC):
                            nc.tensor.matmul(
                                ps[:],
                                q3sb[:, kc * LQ + jc * 128:kc * LQ + (jc + 1) * 128],
                                crsb[:, kc * LC + it * 512:kc * LC + (it + 1) * 512],
                                start=(kc == 0), stop=(kc == HC - 1))
                        nc.scalar.activation(
                            prt[:, jc * LC + it * 512:jc * LC + (it + 1) * 512],
                            ps[:], AF.Exp, bias=cbsb[:, jc:jc + 1])
                    if it > 0:
                        rowsum(it - 1)
                rowsum(IT - 1)

                if xsb is None:
                    xsb = x_block()

                # ---- M2/M4: A^T, Bt^T; evict-normalize on DVE. The last
                # batch splits its DMAs so the final transfer starts sooner.
                def mout(row0, lhs, sec):
                    for hc in range(HC):
                        o = opool.tile([128, LC], F16, tag=f"o{sec}")
                        for it in range(IT):
                            i0, i1 = it * 512, (it + 1) * 512
                            ps_o = mm_ps.tile([128, 512], F32, tag="mm")
                            for jc in range(JC):
                                nc.tensor.matmul(
                                    ps_o[:],
                                    lhs[:, jc * H + hc * 128:jc * H + (hc + 1) * 128],
                                    prt[:, jc * LC + i0:jc * LC + i1],
                                    start=(jc == 0), stop=(jc == JC - 1))
                            nc.vector.tensor_tensor(
                                o[:, i0:i1], ps_o[:], rrep[:, i0:i1], MUL)
                            if b == NB - 1 and it % 2 == 1:
                                nc.sync.dma_start(
                                    out[b, row0 + hc * 128:row0 + (hc + 1) * 128,
                                        i0 - 512:i1], o[:, i0 - 512:i1])
                        if b < NB - 1:
                            nc.sync.dma_start(
                                out[b, row0 + hc * 128:row0 + (hc + 1) * 128, :],
                                o[:])

                mout(0, qtsb, "a")
                mout(H, xsb, "b")

    nc.compile()
    return nc


def _prep(C, Q, cmask, qmask, line_project):
    import ml_dtypes
    w1, w2, w3 = np.split(line_project.astype(np.float64), 3)
    r = np.einsum('bhi,h->bi', C.astype(np.float64), w1).astype(np.float32)
    c_ = np.einsum('bhj,h->bj', Q.astype(np.float64), w2).astype(np.float32)
    # -ln(64) shift keeps exp within fp8 e4m3 range; cancels in col-normalize
    rm = (r - NEG * cmask - np.float32(np.log(64.0))).reshape(
        B, IC, 128).transpose(0, 2, 1)
    cb = (c_ - NEG * qmask).reshape(B, JC, 128).transpose(0, 2, 1)
    rcb = np.concatenate([rm, cb], axis=2).astype(np.float32)

    # fp16 row-path operands
    c16 = np.ascontiguousarray(
        C.reshape(B, HC, 128, LC).transpose(0, 2, 1, 3)).astype(np.float16)
    w3f = w3.astype(np.float32)
    q3v = Q * w3f[None, :, None]
    q3 = np.ascontiguousarray(
        q3v.reshape(B, HC, 128, LQ).transpose(0, 2, 1, 3)).astype(np.float16)

    # fp8 col-path operands: fold 4*sqrt(|w3|) into both sides;
    # S3_dev = 16*S3, undone by the ACT exp scale (1/16).
    sq = 4.0 * np.sqrt(np.abs(w3f))
    c8v = C * sq[None, :, None]
    c8 = np.ascontiguousarray(
        c8v.reshape(B, HC, 128, LC).transpose(0, 2, 1, 3)
    ).astype(ml_dtypes.float8_e4m3)
    q38v = Q * (np.sign(w3f) * sq)[None, :, None]
    q38 = np.ascontiguousarray(
        q38v.reshape(B, HC, 128, LQ).transpose(0, 2, 1, 3)
    ).astype(ml_dtypes.float8_e4m3)

    Ct = C.transpose(0, 2, 1)  # [B, LC, H]
    cta = np.ones((B, 128, IC, HA), dtype=ml_dtypes.float8_e4m3)
    cta[..., :H] = Ct.reshape(B, IC, 128, H).transpose(0, 2, 1, 3).astype(
        ml_dtypes.float8_e4m3)
    qt = np.ascontiguousarray(
        Q.transpose(0, 2, 1).reshape(B, JC, 128, H).transpose(0, 2, 1, 3)
    ).astype(np.float16)
    return rcb, c16, q3, c8, cta, q38, qt


def make_in_maps(C, Q, cmask, qmask, line_project):
    C = np.asarray(C, dtype=np.float32)
    Q = np.asarray(Q, dtype=np.float32)
    cmask = np.asarray(cmask, dtype=np.float32)
    qmask = np.asarray(qmask, dtype=np.float32)
    line_project = np.asarray(line_project, dtype=np.float32)
    rcb, c16, q3, c8, cta, q38, qt = _prep(C, Q, cmask, qmask, line_project)
    in_maps = []
    for core in range(NCORES):
        s = slice(core * NB, (core + 1) * NB)
        in_maps.append({
            "c16": np.ascontiguousarray(c16[s]).reshape(NB, 128, HC * LC),
            "q3": np.ascontiguousarray(q3[s]).reshape(NB, 128, HC * LQ),
            "c8": np.ascontiguousarray(c8[s]).reshape(NB, 128, HC * LC),
            "cta": np.ascontiguousarray(cta[s]).reshape(NB, 128, IC * HA),
            "q38": np.ascontiguousarray(q38[s]).reshape(NB, 128, HC * LQ),
            "qt": np.ascontiguousarray(qt[s]).reshape(NB, 128, JC * H),
            "rcb": np.ascontiguousarray(rcb[s]),
        })
    return in_maps


def kernel(C, Q, cmask, qmask, line_project):
    from concourse.bass_utils import run_bass_kernel_spmd

    C = np.asarray(C, dtype=np.float32)
    in_maps = make_in_maps(C, Q, cmask, qmask, line_project)
    if "nc" not in _CACHE:
        _CACHE["nc"] = _build()
    nc = _CACHE["nc"]
    res = run_bass_kernel_spmd(nc, in_maps, core_ids=list(range(NCORES)))
    _CACHE["last_results"] = res
    dev = np.concatenate([res.results[c]["out"] for c in range(NCORES)], axis=0)
    A = dev[:, :H].astype(np.float32)
    Bt = dev[:, H:].astype(np.float32)
    full = np.empty((B, 4 * H, LC), dtype=np.float32)
    full[:, :H] = C
    full[:, H:2 * H] = A
    full[:, 2 * H:3 * H] = C * A
    full[:, 3 * H:] = C * Bt
    return full
